# revision 39
# baseline (speedup 1.0000x reference)
"""Trainium2 Bass kernel for nn_Attention (B=4, N=2048, C=768, H=12, D=64).

Sharding: core c -> batch b=c//2, head-group hg=c%2 (6 heads each).
qkv_w column-parallel, proj_w row-parallel (host sums the 2 partials per b).
All matmuls in float32r (TF32-like, full PE rate for moving dims >= 256).

Structure (single TileContext program per core), organized so attention for
head-pair p overlaps QK prep for pair p+1:
  - V matmuls for all token tiles first (vA tile, ones column folded in for
    the softmax denominator).
  - Per head-pair p: QK matmuls (host packs wqkvT as [q0k0|q1k1|q2k2|v] so
    each pair is a contiguous 256-wide moving operand), RMSNorm+RoPE via
    host-prepped tables (norm weights and the x8 factor folded in),
    PE-transpose to feature-major; then attention for that pair:
    S^T = K Q^T, exp on ACT (scale=1/8; no max-subtraction needed since
    RMSNorm gives ||q||=||k||=8 so |s|<=8), AV accumulation with the ones
    column giving the denominator; reciprocal + GPSIMD partition_broadcast
    to normalize.
  - Projection per token tile at the end (overlaps the attention tail).
"""
import sys

sys.path.insert(0, "/opt/trn_rl_repo")

import numpy as np
import concourse.bass as bass
import concourse.mybir as mybir
import concourse.tile as tile
from concourse import bacc
from concourse.bass_utils import run_bass_kernel_spmd
from concourse.masks import make_identity

dt = mybir.dt
AF = mybir.ActivationFunctionType
ALU = mybir.AluOpType
AX = mybir.AxisListType

B, N, C = 4, 2048, 768
H, D = 12, 64
HPC = 6            # heads per core
EPS = 1e-6
NT = N // 128      # 16 token tiles
NCHUNK = C // 128  # 6 contraction chunks
SCALE = D ** -0.5  # 0.125
NG = 4             # qi groups
G = N // NG        # 512 per group

DEBUG_DUMP = False


def _bc(ap, idx, count):
    """Insert a broadcast (step 0) free dim at position idx of an AP."""
    a = list(ap.ap)
    a.insert(idx, [0, count])
    return bass.AP(tensor=ap.tensor, offset=ap.offset, ap=a)


def build_program():
    nc = bacc.Bacc(None, target_bir_lowering=False)

    xT = nc.dram_tensor("xT", [C, N], dt.float32r, kind="ExternalInput")
    # host layout: [q0|k0 (256) | q1|k1 | q2|k2 | v (384)]
    wqkvT = nc.dram_tensor("wqkvT", [C, 3 * HPC * D], dt.float32r, kind="ExternalInput")
    projT = nc.dram_tensor("projT", [HPC * D, C], dt.float32r, kind="ExternalInput")
    cqk = nc.dram_tensor("cqk", [N, 2 * D], dt.float32, kind="ExternalInput")
    sqk = nc.dram_tensor("sqk", [N, 2 * D], dt.float32, kind="ExternalInput")
    out = nc.dram_tensor("out", [N, C], dt.float32, kind="ExternalOutput")
    if DEBUG_DUMP:
        dbg_qT0 = nc.dram_tensor("dbg_qT0", [128, N], dt.float32, kind="ExternalOutput")
        dbg_kT0 = nc.dram_tensor("dbg_kT0", [128, N], dt.float32, kind="ExternalOutput")
        dbg_oT0 = nc.dram_tensor("dbg_oT0", [128, N], dt.float32, kind="ExternalOutput")
        dbg_vA = nc.dram_tensor("dbg_vA", [128, NT * HPC * (D + 1)], dt.float32, kind="ExternalOutput")

    with tile.TileContext(nc) as tc:
        with (
            tc.tile_pool(name="persist", bufs=1) as persist,
            tc.tile_pool(name="qkrot", bufs=2) as qkrot,     # qT/kT rotate across pairs
            tc.tile_pool(name="work", bufs=2) as work,
            tc.tile_pool(name="qkblk", bufs=1) as qkblk,
            tc.tile_pool(name="tiny", bufs=2) as tiny,
            tc.tile_pool(name="den", bufs=1) as den,
            tc.tile_pool(name="p2e", bufs=2) as p2e,
            tc.tile_pool(name="psA", bufs=2, space="PSUM") as psA,   # qkv/tp/proj shared slots
            tc.tile_pool(name="psS", bufs=2, space="PSUM") as psS,   # scores (2 banks ea)
            tc.tile_pool(name="psV", bufs=2, space="PSUM") as psV,   # AV accum
        ):
            # ---------------- persistent tiles --------------------------------
            oT = [[persist.tile([128, G], dt.float32r, name=f"oT{p}_{g}", tag=f"oT{p}_{g}")
                   for g in range(NG)] for p in range(3)]
            vA = [persist.tile([128, 4, HPC, D + 1], dt.float32r, name=f"vA{kg}", tag=f"vA{kg}")
                  for kg in range(NG)]
            ident = persist.tile([128, 128], dt.float32, tag="ident")
            make_identity(nc, ident[:])
            ones1 = persist.tile([128, 1], dt.float32, tag="ones1")
            nc.vector.memset(ones1[:], 1.0)
            for kg in range(NG):
                nc.vector.tensor_copy(vA[kg][:, :, :, D : D + 1], _bc(_bc(ones1[:], 1, 4), 2, HPC))

            # weights / x^T / tables
            xw_cm = tc.tile_pool(name="xw", bufs=1)
            xw = xw_cm.__enter__()
            xr = []
            wr = []
            for j in range(NCHUNK):
                xj = xw.tile([128, N], dt.float32r, name=f"xr{j}", tag=f"xr{j}")
                wj = xw.tile([128, 3 * HPC * D], dt.float32r, name=f"wr{j}", tag=f"wr{j}")
                eng = (nc.sync, nc.gpsimd)[j % 2]
                eng.dma_start(xj[:], xT[j * 128 : (j + 1) * 128, :])
                eng.dma_start(wj[:], wqkvT[j * 128 : (j + 1) * 128, :])
                xr.append(xj)
                wr.append(wj)
            tabs = {}
            for name, dram in (("cqk", cqk), ("sqk", sqk)):
                t = persist.tile([128, NT, 2, D], dt.float32, name=name, tag=name)
                nc.gpsimd.dma_start(t[:], dram.rearrange("(t p) (qk d) -> p t qk d", p=128, qk=2))
                tabs[name] = t
            prW = []
            for p in range(3):
                wp = persist.tile([128, C], dt.float32r, name=f"prW{p}", tag=f"prW{p}")
                nc.gpsimd.dma_start(wp[:], projT[p * 128 : (p + 1) * 128, :])
                prW.append(wp)

            # ------- interleaved emission: prep / attention / projection ------
            # Engines execute their instruction streams in order, so emission
            # order IS the schedule. Pair p's attention units interleave the
            # prep-tile emission for pair p+1 (2 tiles per unit) so the PE
            # stream mixes prep matmuls with scores/AV instead of bunching
            # them at pair boundaries. Pair-2 attention interleaves the
            # projection of already-finished qi groups.

            def new_pair_state(p):
                return {
                    "p": p,
                    "qT": [qkrot.tile([128, G], dt.float32r, name=f"qT{p}_{g}", tag=f"qT{g}") for g in range(NG)],
                    "kT": [qkrot.tile([128, G], dt.float32r, name=f"kT{p}_{g}", tag=f"kT{g}") for g in range(NG)],
                    "pend": [],
                    "next": 0,
                }

            def flush_one(st):
                i, qn = st["pend"].pop(0)
                for half, dstT in ((0, st["qT"]), (1, st["kT"])):
                    tp = psA.tile([128, 128], dt.float32, tag="qkv")
                    nc.tensor.transpose(tp[:], qn[:, half * 128 : (half + 1) * 128], ident[:])
                    dst = dstT[i // NG][:, (i % NG) * 128 : (i % NG + 1) * 128]
                    if st["p"] == 0:
                        nc.scalar.copy(dst, tp[:])
                    else:
                        nc.vector.tensor_copy(dst, tp[:])

            def emit_prep_tile(st):
                p = st["p"]
                i = st["next"]
                st["next"] += 1
                if p == 0:
                    vp = psA.tile([128, HPC * D], dt.float32, tag="qkv")
                    for j in range(NCHUNK):
                        nc.tensor.matmul(vp[:], xr[j][:, i * 128 : (i + 1) * 128],
                                         wr[j][:, 6 * 128 : 6 * 128 + HPC * D],
                                         start=(j == 0), stop=(j == NCHUNK - 1))
                    nc.scalar.copy(vA[i // NG][:, i % NG, :, 0:D], vp[:].rearrange("p (h d) -> p h d", h=HPC))
                qkp = psA.tile([128, 256], dt.float32, tag="qkv")
                for j in range(NCHUNK):
                    nc.tensor.matmul(qkp[:], xr[j][:, i * 128 : (i + 1) * 128],
                                     wr[j][:, p * 256 : (p + 1) * 256],
                                     start=(j == 0), stop=(j == NCHUNK - 1))
                if len(st["pend"]) >= 2:
                    flush_one(st)
                qk_sb = qkblk.tile([128, 256], dt.float32, tag=f"qk_sb{i % 4}")
                if p == 0:
                    nc.scalar.copy(qk_sb[:], qkp[:])
                else:
                    nc.vector.tensor_copy(qk_sb[:], qkp[:])
                qk4 = qk_sb[:].rearrange("p (h d) -> p h d", h=4)
                sq = work.tile([128, 4, D], dt.float32, tag="m2")
                nc.vector.tensor_tensor(sq[:], qk4, qk4, op=ALU.mult)
                ss = tiny.tile([128, 4], dt.float32, tag="ss16")
                nc.vector.tensor_reduce(ss[:], sq[:], axis=AX.X, op=ALU.add)
                # rsqrt on DVE (bit-trick + 2 Newton): nf = 1/sqrt(ss+D*EPS)
                ssh = tiny.tile([128, 4], dt.float32, tag="ssh")
                nc.vector.tensor_scalar(ssh[:], ss[:], 0.5, 0.5 * D * EPS,
                                        op0=ALU.mult, op1=ALU.add)
                y0i = tiny.tile([128, 4], dt.int32, tag="y0i")
                nc.vector.tensor_scalar(y0i[:], ss[:].bitcast(dt.int32), 1, 0,
                                        op0=ALU.logical_shift_right, op1=ALU.bitwise_or)
                nc.vector.tensor_scalar(y0i[:], y0i[:], -1, 0x5F3759DF,
                                        op0=ALU.mult, op1=ALU.add)
                nf16 = tiny.tile([128, 4], dt.float32, tag="nf16")
                y1 = tiny.tile([128, 4], dt.float32, tag="y1")
                yw = tiny.tile([128, 4], dt.float32, tag="yw")
                y = y0i[:].bitcast(dt.float32)
                for dst_ in (y1, nf16):
                    nc.vector.tensor_tensor(yw[:], y, y, op=ALU.mult)
                    nc.vector.tensor_tensor(yw[:], yw[:], ssh[:], op=ALU.mult)
                    nc.vector.tensor_scalar(yw[:], yw[:], -1.0, 1.5,
                                            op0=ALU.mult, op1=ALU.add)
                    nc.vector.tensor_tensor(dst_[:], y, yw[:], op=ALU.mult)
                    y = dst_[:]
                nfb = _bc(nf16[:], 2, D)
                t_ = work.tile([128, 4, D], dt.float32, tag="t_")
                nc.vector.tensor_tensor(t_[:], qk4, nfb, op=ALU.mult)
                # tables: [128, NT, 2(qk), D] with heads broadcast
                cwb = _bc(tabs["cqk"][:, i, :, :], 2, 2)
                swb = _bc(tabs["sqk"][:, i, :, :], 2, 2)
                t4 = t_[:].rearrange("p (qk h) d -> p qk h d", qk=2)
                m1 = work.tile([128, 2, 2, D], dt.float32, tag="m1")
                nc.vector.tensor_tensor(m1[:], t4, cwb, op=ALU.mult)
                m2 = work.tile([128, 2, 2, D], dt.float32, tag="m2")
                h_ = D // 2
                nc.gpsimd.tensor_tensor(m2[:, :, :, 0:h_], t4[:, :, :, h_:D], swb[:, :, :, 0:h_], op=ALU.mult)
                nc.gpsimd.tensor_tensor(m2[:, :, :, h_:D], t4[:, :, :, 0:h_], swb[:, :, :, h_:D], op=ALU.mult)
                qn = work.tile([128, 256], dt.float32, tag="qn", bufs=4)
                nc.gpsimd.tensor_tensor(qn[:].rearrange("p (qk h d) -> p qk h d", qk=2, h=2), m1[:], m2[:], op=ALU.add)
                st["pend"].append((i, qn))

            def finish_prep(st):
                while st["pend"]:
                    flush_one(st)

            def emit_att_unit(st, g, hh):
                p = st["p"]
                h = 2 * p + hh
                off = 64 * hh
                av = psV.tile([65, G], dt.float32, tag="av")

                def emit_av(kpair, es):
                    for half in range(2):
                        ki = kpair * 2 + half
                        nc.tensor.matmul(
                            av[:],
                            vA[ki // NG][:, ki % NG, h, :],
                            es[:, half * 512 : (half + 1) * 512],
                            start=(ki == 0), stop=(ki == NT - 1),
                        )

                # AV trails exp by one kpair so PE never stalls on the
                # in-flight ACTIVATE.
                prev = None
                for kpair in range(8):
                    sp = psS.tile([128, 1024], dt.float32, tag="sp")
                    for half in range(2):
                        ki = kpair * 2 + half
                        nc.tensor.matmul(
                            sp[:, half * 512 : (half + 1) * 512],
                            st["kT"][ki // NG][off : off + 64, (ki % NG) * 128 : (ki % NG + 1) * 128],
                            st["qT"][g][off : off + 64, :],
                            start=True, stop=True,
                        )
                    es = p2e.tile([128, 1024], dt.float32r, tag="es")
                    nc.scalar.activation(es[:], sp[:], AF.Exp, scale=SCALE)
                    if prev is not None:
                        emit_av(*prev)
                    prev = (kpair, es)
                emit_av(*prev)
                rd = den.tile([1, G], dt.float32, tag="rd")
                nc.vector.reciprocal(rd[:], av[64:65, :])
                bc = den.tile([64, G], dt.float32, tag="bc")
                nc.gpsimd.partition_broadcast(bc[:], rd[:], channels=64)
                nc.vector.tensor_tensor(
                    oT[p][g][off : off + 64, :],
                    av[0:64, :], bc[:], op=ALU.mult,
                )

            def emit_proj_tile(i):
                p512 = psA.tile([128, 512], dt.float32, tag="qkv")
                p256 = psA.tile([128, 256], dt.float32, tag="qkv")
                for pp_ in range(3):
                    st_, spp = (pp_ == 0), (pp_ == 2)
                    sl = oT[pp_][i // NG][:, (i % NG) * 128 : (i % NG + 1) * 128]
                    nc.tensor.matmul(p512[:], sl, prW[pp_][:, 0:512], start=st_, stop=spp)
                    nc.tensor.matmul(p256[:], sl, prW[pp_][:, 512:768], start=st_, stop=spp)
                os_ = outp.tile([128, C], dt.float32, tag="os")
                nc.vector.tensor_copy(os_[:, 0:512], p512[:])
                nc.vector.tensor_copy(os_[:, 512:768], p256[:])
                nc.sync.dma_start(out[i * 128 : (i + 1) * 128, :], os_[:])

            # pair-0 prep up front (V matmuls included)
            cur = new_pair_state(0)
            for _ in range(NT):
                emit_prep_tile(cur)
            finish_prep(cur)

            outp = None
            proj_queue = list(range(NT))
            for p in range(3):
                nxt = new_pair_state(p + 1) if p < 2 else None
                if p == 2:
                    # x^T / qkv weights are dead after pair-2 prep; free them
                    # and open the output pool so projection can interleave.
                    xw_cm.__exit__(None, None, None)
                    outp_cm = tc.tile_pool(name="outp", bufs=2)
                    outp = outp_cm.__enter__()
                for g in range(NG):
                    for hh in range(2):
                        emit_att_unit(cur, g, hh)
                        if nxt is not None:
                            emit_prep_tile(nxt)
                            emit_prep_tile(nxt)
                        else:
                            budget = 2
                            while budget and proj_queue and proj_queue[0] < g * NG:
                                emit_proj_tile(proj_queue.pop(0))
                                budget -= 1
                if nxt is not None:
                    finish_prep(nxt)
                    cur = nxt
            for i in proj_queue:
                emit_proj_tile(i)
            outp_cm.__exit__(None, None, None)

            if DEBUG_DUMP:
                for g in range(NG):
                    nc.sync.dma_start(dbg_oT0[:, g * G : (g + 1) * G], oT[0][g][:].bitcast(dt.float32))
                for kg in range(NG):
                    nc.sync.dma_start(dbg_vA[:, kg * 4 * HPC * (D + 1) : (kg + 1) * 4 * HPC * (D + 1)],
                                      vA[kg][:].bitcast(dt.float32).rearrange("p a b c -> p (a b c)"))

    nc.compile()
    return nc


_NC = None


def _get_nc():
    global _NC
    if _NC is None:
        _NC = build_program()
    return _NC


def _prep_inputs(x, cos, sin, qkv_w, q_norm_w, k_norm_w, proj_w):
    cos2 = np.asarray(cos, np.float32).reshape(N, D // 2)
    sin2 = np.asarray(sin, np.float32).reshape(N, D // 2)
    cos_full = np.concatenate([cos2, cos2], axis=1)          # [N, 64]
    sin_signed = np.concatenate([-sin2, sin2], axis=1)       # [N, 64]

    def tables(w):
        w = np.asarray(w, np.float32)
        wswap = np.concatenate([w[D // 2 :], w[: D // 2]])
        cw = (8.0 * cos_full * w[None, :]).astype(np.float32)
        sw = (8.0 * sin_signed * wswap[None, :]).astype(np.float32)
        return np.ascontiguousarray(cw), np.ascontiguousarray(sw)

    cwq_, swq_ = tables(q_norm_w)
    cwk_, swk_ = tables(k_norm_w)
    cqk_ = np.ascontiguousarray(np.stack([cwq_, cwk_], axis=1).reshape(N, 2 * D))
    sqk_ = np.ascontiguousarray(np.stack([swq_, swk_], axis=1).reshape(N, 2 * D))

    in_maps = []
    for c in range(8):
        b, hg = c // 2, c % 2
        h0 = HPC * hg
        rows = np.r_[h0 * D : (h0 + HPC) * D]
        wq = qkv_w[rows]          # [384, C]
        wk = qkv_w[C + rows]
        wv = qkv_w[2 * C + rows]
        # pack as [q0|k0, q1|k1, q2|k2, v]
        parts = []
        for p in range(3):
            parts.append(wq[p * 128 : (p + 1) * 128])
            parts.append(wk[p * 128 : (p + 1) * 128])
        parts.append(wv)
        wqkvT_ = np.ascontiguousarray(np.concatenate(parts, 0).T, np.float32)
        projT_ = np.ascontiguousarray(proj_w[:, rows].T, np.float32)
        xT_ = np.ascontiguousarray(x[b].T, np.float32)
        in_maps.append({
            "xT": xT_, "wqkvT": wqkvT_, "projT": projT_,
            "cqk": cqk_, "sqk": sqk_,
        })
    return in_maps


def kernel(x, cos, sin, qkv_w, q_norm_w, k_norm_w, proj_w, proj_b, _want_trace=False):
    x = np.asarray(x, np.float32)
    qkv_w = np.asarray(qkv_w, np.float32)
    proj_w = np.asarray(proj_w, np.float32)
    proj_b = np.asarray(proj_b, np.float32)
    in_maps = _prep_inputs(x, cos, sin, qkv_w, q_norm_w, k_norm_w, proj_w)
    nc = _get_nc()
    res = run_bass_kernel_spmd(nc, in_maps, core_ids=list(range(8)), trace=_want_trace)
    out = np.empty((B, N, C), np.float32)
    for b in range(B):
        out[b] = res.results[2 * b]["out"] + res.results[2 * b + 1]["out"] + proj_b[None, :]
    if _want_trace:
        return out, res
    return out
